# revision 4
# baseline (speedup 1.0000x reference)
"""LIF spike kernel (T=4 scan with threshold reset) on 8 TRN2 NeuronCores.

Recurrence per element (tau=1, thresh=1):
    s_t     = m_{t-1} + x_t
    spike_t = (s_t > 1)           -> output
    m_t     = s_t * (s_t <= 1)    -> threshold reset

Sharding: pure data-parallel over the batch axis (dim 1, 64 -> 8 per core).

Per-core DMA is the roofline: 16 SDMA engines x ~27 GB/s = ~435 GB/s
aggregate (SBUF AXI ports). f32 loads are irreducible (16.8 MB), but spikes
are 0/1, so they are stored as int8 sign values (4.2 MB): the ACT engine
computes spike = sign(s_t - 1) straight into an int8 tile (exact: s-1 is
Sterbenz-exact near 1, so sign(s-1)==+1 iff s>1) and the host maps
(stored == 1) -> f32. DMA floor: 21 MB / 435 GB/s ~= 48 us.

Compute is split so no engine exceeds that floor (f32 2-input DVE ops run
at 1x only, ~1.2 us per [128,1024] tile):
  DVE : 3 adds + 2 resets            ~6.1 us/chunk  (the scan chain)
  Pool: t=0 reset as is_le + mult    ~3.5 us/chunk  (STT unsupported on Pool)
  ACT : 4x sign -> int8              ~3.4 us/chunk
All outputs go to distinct tiles (no WAR serialization); loads ride the SP
HWDGE ring, the single int8 store per chunk rides the ACT HWDGE ring.
"""

import numpy as np

import concourse.bacc as bacc
import concourse.mybir as mybir
import concourse.tile as tile
from concourse import bass_utils

T = 4
B_FULL = 64
C, H, W = 128, 32, 32
N_CORES = 8
B_LOC = B_FULL // N_CORES            # 8
N = B_LOC * C * H * W                # 1048576 elements per core per timestep
P = 128                              # SBUF partitions

_LE = mybir.AluOpType.is_le
_MUL = mybir.AluOpType.mult
_ADD = mybir.AluOpType.add
_SIGN = mybir.ActivationFunctionType.Sign

F = 1024
BUFS = 3

_nc_cache = None


def _build(F=F, bufs=BUFS):
    nchunk = N // (P * F)
    nc = bacc.Bacc(
        "TRN2",
        target_bir_lowering=False,
        debug=False,
        enable_asserts=False,
    )
    x_d = nc.dram_tensor("x", [T, N], mybir.dt.float32, kind="ExternalInput").ap()
    y_d = nc.dram_tensor(
        "y", [nchunk, P, T * F], mybir.dt.int8, kind="ExternalOutput"
    ).ap()
    # [t, n, p, f] view of the flat [T, N] input
    xv = x_d.rearrange("t (n p f) -> t n p f", p=P, f=F)
    # activation() lowers a float bias to a const AP; -1.0 isn't in the
    # pre-registered set, so register it (same recipe as bass __init__)
    neg1 = nc.alloc_sbuf_tensor("const-float32--1.0", [128, 1], mybir.dt.float32)
    nc.gpsimd.memset(neg1.ap(), -1.0)
    nc.const_aps.aps[(mybir.dt.float32, -1.0)] = neg1.ap()

    with tile.TileContext(nc) as tc:
        with (
            tc.tile_pool(name="xin", bufs=bufs) as xp,
            tc.tile_pool(name="spk", bufs=bufs) as spp,
            tc.tile_pool(name="wrk", bufs=bufs) as wkp,
        ):
            for j in range(nchunk):
                xt = []
                for t in range(T):
                    xtile = xp.tile(
                        [P, F], mybir.dt.float32, tag=f"x{t}", name=f"x{t}_{j}"
                    )
                    nc.sync.dma_start(xtile[:], xv[t, j])
                    xt.append(xtile[:])
                spall = spp.tile([P, T * F], mybir.dt.int8, tag="s", name=f"s_{j}")
                sp = [spall[:, t * F : (t + 1) * F] for t in range(T)]

                def wtile(tag):
                    return wkp.tile(
                        [P, F], mybir.dt.float32, tag=tag, name=f"{tag}_{j}"
                    )[:]

                # t=0: spike straight from x0 (ACT); reset on Pool (2 ops)
                nc.scalar.activation(sp[0], xt[0], _SIGN, bias=-1.0)
                mask = wtile("mask")
                m0 = wtile("m0")
                nc.gpsimd.tensor_single_scalar(mask, xt[0], 1.0, _LE)
                nc.gpsimd.tensor_tensor(m0, mask, xt[0], _MUL)

                # t=1..2: add (DVE) -> sign (ACT) + reset (DVE)
                m = m0
                for t in range(1, T - 1):
                    s = wtile(f"s{t}")
                    nc.vector.tensor_tensor(s, m, xt[t], _ADD)
                    nc.scalar.activation(sp[t], s, _SIGN, bias=-1.0)
                    m = wtile(f"m{t}")
                    nc.vector.scalar_tensor_tensor(m, s, 1.0, s, _LE, _MUL)

                # t=3: add + sign only (last membrane is dead)
                s = wtile("s3")
                nc.vector.tensor_tensor(s, m, xt[T - 1], _ADD)
                nc.scalar.activation(sp[T - 1], s, _SIGN, bias=-1.0)

                nc.scalar.dma_start(y_d[j], spall[:])

    nc.compile()
    return nc


def _get_nc():
    global _nc_cache
    if _nc_cache is None:
        _nc_cache = _build()
    return _nc_cache


def _run(x, **spmd_kwargs):
    x = np.asarray(x, dtype=np.float32)
    assert x.shape == (T, B_FULL, C, H, W), x.shape
    nchunk = N // (P * F)
    in_maps = [
        {
            "x": np.ascontiguousarray(
                x[:, c * B_LOC : (c + 1) * B_LOC]
            ).reshape(T, N)
        }
        for c in range(N_CORES)
    ]
    res = bass_utils.run_bass_kernel_spmd(
        _get_nc(), in_maps, core_ids=list(range(N_CORES)), **spmd_kwargs
    )
    out = np.empty((T, B_FULL, C, H, W), dtype=np.float32)
    for c in range(N_CORES):
        y = res.results[c]["y"]  # [nchunk, P, T*F] int8, sign in {-1,0,1}
        spikes = (
            y.reshape(nchunk, P, T, F).transpose(2, 0, 1, 3).reshape(T, N) == 1
        )
        out[:, c * B_LOC : (c + 1) * B_LOC] = spikes.reshape(
            T, B_LOC, C, H, W
        )
    return out, res


def kernel(x):
    out, _ = _run(x)
    return out


# revision 5
# speedup vs baseline: 2.5254x; 2.5254x over previous
"""LIF spike kernel (T=4 scan with threshold reset) on 8 TRN2 NeuronCores.

Recurrence per element (tau=1, thresh=1):
    s_t     = m_{t-1} + x_t
    spike_t = (s_t > 1)           -> output
    m_t     = s_t * (s_t <= 1)    -> threshold reset

Sharding: pure data-parallel over the batch axis (dim 1, 64 -> 8 per core).

Per-core DMA is the roofline: 16 SDMA engines x ~27 GB/s = ~435 GB/s
aggregate (SBUF AXI ports). f32 loads are irreducible (16.8 MB), but spikes
are 0/1, so they are stored as int8 sign values (4.2 MB): the ACT engine
computes spike = sign(s_t - 1) straight into an int8 tile (exact: s-1 is
Sterbenz-exact near 1, so sign(s-1)==+1 iff s>1) and the host maps
(stored == 1) -> f32. DMA floor: 21 MB / 435 GB/s ~= 48 us.

Compute is split so no engine exceeds that floor (f32 2-input DVE ops run
at 1x only, ~1.2 us per [128,1024] tile):
  DVE : 3 adds + 2 resets            ~6.1 us/chunk  (the scan chain)
  Pool: t=0 reset as is_le + mult    ~3.5 us/chunk  (STT unsupported on Pool)
  ACT : 4x sign -> int8              ~3.4 us/chunk
All outputs go to distinct tiles (no WAR serialization); loads ride the SP
HWDGE ring, the single int8 store per chunk rides the ACT HWDGE ring.
"""

import numpy as np

import concourse.bacc as bacc
import concourse.mybir as mybir
import concourse.tile as tile
from concourse import bass_utils

T = 4
B_FULL = 64
C, H, W = 128, 32, 32
N_CORES = 8
B_LOC = B_FULL // N_CORES            # 8
N = B_LOC * C * H * W                # 1048576 elements per core per timestep
P = 128                              # SBUF partitions

_LE = mybir.AluOpType.is_le
_MUL = mybir.AluOpType.mult
_ADD = mybir.AluOpType.add
_SIGN = mybir.ActivationFunctionType.Sign

F = 1024
BUFS = 3

_nc_cache = None


def _build(F=F, bufs=BUFS):
    nchunk = N // (P * F)
    nc = bacc.Bacc(
        "TRN2",
        target_bir_lowering=False,
        debug=False,
        enable_asserts=False,
    )
    x_d = nc.dram_tensor("x", [T, N], mybir.dt.float32, kind="ExternalInput").ap()
    y_d = nc.dram_tensor(
        "y", [nchunk, P, T * F], mybir.dt.int8, kind="ExternalOutput"
    ).ap()
    # [t, n, p, f] view of the flat [T, N] input
    xv = x_d.rearrange("t (n p f) -> t n p f", p=P, f=F)
    # activation() lowers a float bias to a const AP; -1.0 isn't in the
    # pre-registered set, so register it (same recipe as bass __init__)
    neg1 = nc.alloc_sbuf_tensor("const-float32--1.0", [128, 1], mybir.dt.float32)
    nc.gpsimd.memset(neg1.ap(), -1.0)
    nc.const_aps.aps[(mybir.dt.float32, -1.0)] = neg1.ap()

    with tile.TileContext(nc) as tc:
        with (
            tc.tile_pool(name="xin", bufs=bufs) as xp,
            tc.tile_pool(name="spk", bufs=bufs) as spp,
            tc.tile_pool(name="wrk", bufs=bufs) as wkp,
        ):
            for j in range(nchunk):
                xt = []
                for t in range(T):
                    xtile = xp.tile(
                        [P, F], mybir.dt.float32, tag=f"x{t}", name=f"x{t}_{j}"
                    )
                    nc.sync.dma_start(xtile[:], xv[t, j])
                    xt.append(xtile[:])
                spall = spp.tile([P, T * F], mybir.dt.int8, tag="s", name=f"s_{j}")
                sp = [spall[:, t * F : (t + 1) * F] for t in range(T)]

                def wtile(tag):
                    return wkp.tile(
                        [P, F], mybir.dt.float32, tag=tag, name=f"{tag}_{j}"
                    )[:]

                # t=0: spike straight from x0 (ACT); reset on DVE.
                # (Pool compute measured 13x slower than modeled AND its SBUF
                # port traffic slowed DVE ops 3.7x -- keep GpSimd idle.)
                nc.scalar.activation(sp[0], xt[0], _SIGN, bias=-1.0)
                m0 = wtile("m0")
                nc.vector.scalar_tensor_tensor(m0, xt[0], 1.0, xt[0], _LE, _MUL)

                # t=1..2: add (DVE) -> sign (ACT) + reset (DVE)
                m = m0
                for t in range(1, T - 1):
                    s = wtile(f"s{t}")
                    nc.vector.tensor_tensor(s, m, xt[t], _ADD)
                    nc.scalar.activation(sp[t], s, _SIGN, bias=-1.0)
                    m = wtile(f"m{t}")
                    nc.vector.scalar_tensor_tensor(m, s, 1.0, s, _LE, _MUL)

                # t=3: add + sign only (last membrane is dead)
                s = wtile("s3")
                nc.vector.tensor_tensor(s, m, xt[T - 1], _ADD)
                nc.scalar.activation(sp[T - 1], s, _SIGN, bias=-1.0)

                nc.scalar.dma_start(y_d[j], spall[:])

    nc.compile()
    return nc


def _get_nc():
    global _nc_cache
    if _nc_cache is None:
        _nc_cache = _build()
    return _nc_cache


def _run(x, **spmd_kwargs):
    x = np.asarray(x, dtype=np.float32)
    assert x.shape == (T, B_FULL, C, H, W), x.shape
    nchunk = N // (P * F)
    in_maps = [
        {
            "x": np.ascontiguousarray(
                x[:, c * B_LOC : (c + 1) * B_LOC]
            ).reshape(T, N)
        }
        for c in range(N_CORES)
    ]
    res = bass_utils.run_bass_kernel_spmd(
        _get_nc(), in_maps, core_ids=list(range(N_CORES)), **spmd_kwargs
    )
    out = np.empty((T, B_FULL, C, H, W), dtype=np.float32)
    for c in range(N_CORES):
        y = res.results[c]["y"]  # [nchunk, P, T*F] int8, sign in {-1,0,1}
        spikes = (
            y.reshape(nchunk, P, T, F).transpose(2, 0, 1, 3).reshape(T, N) == 1
        )
        out[:, c * B_LOC : (c + 1) * B_LOC] = spikes.reshape(
            T, B_LOC, C, H, W
        )
    return out, res


def kernel(x):
    out, _ = _run(x)
    return out
